# revision 27
# baseline (speedup 1.0000x reference)
"""GemmaAttention (B=2, S=2048, D=2048, H=8, KV=1, HD=256) on 8 trn2 NeuronCores.

Sharding: DP=2 over batch x TP=4 over head-pairs. Core c handles batch c//4 and
heads {2*(c%4), 2*(c%4)+1}. Each core computes its partial o_proj output
(row-parallel Wo); the host sums the 4 partials per batch (the all-reduce is
folded into the host-side unshard).

All matmuls run in bf16 (fp32 PSUM accumulate): fp32r streams at ~half the
bf16 column rate on the PE (389ns vs 213ns per N=512 matmul), so bf16 nearly
doubles tensor-engine throughput. rel-err budget is 2e-2; bf16 lands ~5e-3.

Dataflow per core:
  phase A (projections, per 512-col s-block):
    QT[dq,s], KT[dk,s] accumulate c-major over 16 D-chunks (6 PSUM banks),
    drained via ACT copy (psum->bf16 SBUF, frees banks fast) then RoPE on DVE
    in bf16 (2x mode). V[s,dv] computed directly (lhsT = hT chunk) in 128-row
    chains on 2 more banks -- this PE work covers the RoPE drain latency so
    the PE never idles at s-block boundaries. hT and cos/sin are packed
    s-block-major so each block's DMA completes before the PE needs it (the
    previous [P,DCH,S-512] packing interleaved blocks 1-3 within each DMA, so
    block 1 only landed when the whole 6MB stream did -> 8us PE stall).
  phase C (attention, per (head, q-block) item, software-pipelined):
    scoresT[k,q] = KT_chunk.T @ QT with a skew-2 pipeline: S(i) issues two
    iterations ahead of DEN(i)/AV(i) so the ACT exp (+DVE causal staircase)
    latency is hidden behind PE work. Denominators accumulate on the DVE
    (bf16 adds of the exp tiles into dacc), freeing the PE of the old
    ONEC-matmul reduction; one ONESB matmul per item then reduces+broadcasts
    dacc to all 128 partitions in a single 512-col pass. Per-item
    normalization is deferred one item (reciprocal_approx_fast + DVE scale)
    and o_proj quarters are injected into the next item's PE stream.
"""

import numpy as np
import ml_dtypes

import concourse.bass as bass
import concourse.tile as tile
import concourse.mybir as mybir
from concourse import bacc
from concourse.bass_utils import run_bass_kernel_spmd

P = 128
B, S, D = 2, 2048, 2048
H, KV, HD = 8, 1, 256
ROPE_BASE = 10000.0

HEADS_PER_CORE = 2
DQ = HEADS_PER_CORE * HD          # 512 q-dims per core
DCH = D // P                      # 16 contraction chunks
SBLK = 512                        # s-tile for projection rhs / q-tile
NSBLK = S // SBLK                 # 4
NKC = S // P                      # 16 key chunks
NQCH = DQ // P                    # 4 QT partition chunks
NKCH = HD // P                    # 2 KT partition chunks

F32 = mybir.dt.float32
BF16 = mybir.dt.bfloat16
EXP = mybir.ActivationFunctionType.Exp

LAST_EXEC_TIME_NS = None

_BUILD_CACHE = {}


def _build(causal: bool):
    nc = bacc.Bacc()

    # packed layouts: hTp/csb are s-block-major so each 512-col block's data
    # is one contiguous >=4KB-per-partition DMA (1KB lines cap the DMA ring
    # at ~150 GB/s; 2-3KB+ lines reach ~300-390 GB/s) and blocks arrive in
    # the order the PE consumes them
    hTp = nc.declare_dram_parameter("hTp", [P, NSBLK, DCH, SBLK], BF16,
                                    isOutput=False)
    wqp = nc.declare_dram_parameter("wqp", [P, DCH, DQ], BF16, isOutput=False)
    wkp = nc.declare_dram_parameter("wkp", [P, DCH, HD], BF16, isOutput=False)
    wvp = nc.declare_dram_parameter("wvp", [P, DCH, HD], BF16, isOutput=False)
    wo = nc.declare_dram_parameter("wo", [DQ, D], BF16, isOutput=False)
    csb = nc.declare_dram_parameter("csb", [P, NSBLK, 2, NKCH, SBLK], BF16,
                                    isOutput=False)
    onesb = nc.declare_dram_parameter("onesb", [P, P], BF16, isOutput=False)
    if causal:
        stair = nc.declare_dram_parameter("stair", [P, 2 * SBLK], BF16,
                                          isOutput=False)
    else:
        maskT = nc.declare_dram_parameter("emaskT", [S, S], BF16, isOutput=False)
    outp = nc.declare_dram_parameter("out_partial", [S, D], BF16, isOutput=True)

    from contextlib import ExitStack
    with tile.TileContext(nc) as tc, ExitStack() as ctx:
        pq = ctx.enter_context(tc.tile_pool(name="pq", bufs=1))
        QT = pq.tile([P, NQCH, S], BF16, name="QT")
        KT = pq.tile([P, NKCH, S], BF16, name="KT")
        VN = pq.tile([P, NKC, HD], BF16, name="VN")
        OUTN = pq.tile([P, NQCH, S], BF16, name="OUTN")
        WO = pq.tile([P, NQCH, D], BF16, name="WO")
        ONESB = pq.tile([P, P], BF16, name="ONESB")
        if causal:
            STAIR = pq.tile([P, 2 * SBLK], BF16, name="STAIR")

        # ---- phase A: projections + RoPE -----------------------------
        # pcs/pqt/ptmp live in the outer scope: the phase-A pool close then
        # only waits on PE-side reads (HTS/WQ), not on block 3's RoPE still
        # running on the DVE, so phase C's first scores start immediately
        pcs = ctx.enter_context(tc.tile_pool(name="pcs", bufs=1))
        pqt = ctx.enter_context(tc.tile_pool(name="pqt", bufs=8))
        ptmp = ctx.enter_context(tc.tile_pool(name="ptmp", bufs=6))
        with tc.tile_pool(name="pw", bufs=1) as pw, \
             tc.tile_pool(name="pht", bufs=1) as pht, \
             tc.tile_pool(name="ppqk", bufs=6, space="PSUM") as ppqk, \
             tc.tile_pool(name="ppv", bufs=2, space="PSUM") as ppv:
            WQ = pw.tile([P, DCH, DQ], BF16, name="WQ")
            WK = pw.tile([P, DCH, HD], BF16, name="WK")
            WV = pw.tile([P, DCH, HD], BF16, name="WV")

            HTS = pht.tile([P, NSBLK, DCH, SBLK], BF16, name="ht")
            CS = pcs.tile([P, NSBLK, 2, NKCH, SBLK], BF16, name="cs")
            for sb in range(NSBLK):
                if sb == 0:
                    # demand-ordered start: chunk 0/1 stream as singles so
                    # the first matmul issues ASAP, then 4-chunk groups (4KB
                    # lines) interleaved with the weight groups. Fewer DMAs
                    # matter: each dma_start costs ~650ns of issue time and
                    # hw queues throttle at 3-4 outstanding, so a long small-
                    # DMA stream delays block 1's issue past its deadline.
                    # WV streams late: the V chains only run at the end of
                    # the block (~35us), so V weights never displace the
                    # chunk-critical hT/WQ/WK bytes
                    # the first two issues ride the gpsimd queue: it
                    # exits the startup barrier ~2us before Sync finishes
                    # its preamble and carries nothing else this early, so
                    # chunk 0 lands sooner and the first matmul fires early
                    nc.gpsimd.dma_start(out=HTS[:, 0, 0:1, :],
                                        in_=hTp[:, 0, 0:1, :])
                    nc.gpsimd.dma_start(out=WQ[:, 0:1, :], in_=wqp[:, 0:1, :])
                    nc.sync.dma_start(out=WK[:, 0:8, :], in_=wkp[:, 0:8, :])
                    nc.sync.dma_start(out=HTS[:, 0, 1:2, :],
                                      in_=hTp[:, 0, 1:2, :])
                    nc.sync.dma_start(out=WQ[:, 1:2, :], in_=wqp[:, 1:2, :])
                    nc.sync.dma_start(out=HTS[:, 0, 2:4, :],
                                      in_=hTp[:, 0, 2:4, :])
                    nc.sync.dma_start(out=WQ[:, 2:4, :], in_=wqp[:, 2:4, :])
                    for g in range(3):
                        gs = slice(4 * g + 4, 4 * g + 8)
                        nc.sync.dma_start(out=HTS[:, 0, gs, :],
                                          in_=hTp[:, 0, gs, :])
                        nc.sync.dma_start(out=WQ[:, gs, :], in_=wqp[:, gs, :])
                        if g == 0:
                            nc.sync.dma_start(out=WK[:, 8:16, :],
                                              in_=wkp[:, 8:16, :])
                        if g == 1:
                            nc.sync.dma_start(out=WV[:, 0:8, :],
                                              in_=wvp[:, 0:8, :])
                        if g == 2:
                            nc.sync.dma_start(out=WV[:, 8:16, :],
                                              in_=wvp[:, 8:16, :])
                    nc.sync.dma_start(out=CS[:, 0], in_=csb[:, 0])
                    # blocks 1-3 follow in block-major order, cos/sin after
                    # each block's hT so every block is fully resident before
                    # the PE reaches it
                    for b in range(1, NSBLK):
                        nc.sync.dma_start(out=HTS[:, b], in_=hTp[:, b])
                        nc.sync.dma_start(out=CS[:, b], in_=csb[:, b])
                        if b == 1:
                            nc.sync.dma_start(out=ONESB, in_=onesb[:, :])
                            if causal:
                                nc.sync.dma_start(out=STAIR, in_=stair[:, :])
                if sb == NSBLK - 1:
                    for cc in range(NQCH):
                        # o_proj weights are first needed deep into phase C;
                        # stream them late so they never delay hT/cos/sin
                        nc.sync.dma_start(out=WO[:, cc, :],
                                          in_=wo[cc * P:(cc + 1) * P, :])

                # Q/K accumulation, c-major (DMA-friendly: each ht used
                # 6x right after it lands)
                ssl = slice(sb * SBLK, (sb + 1) * SBLK)
                psq = [ppqk.tile([P, SBLK], F32, name="pp") for _ in range(NQCH)]
                psk = [ppqk.tile([P, SBLK], F32, name="pp") for _ in range(NKCH)]
                for c in range(DCH):
                    for i in range(NQCH):
                        nc.tensor.matmul(psq[i], lhsT=WQ[:, c, i * P:(i + 1) * P],
                                         rhs=HTS[:, sb, c, :], start=(c == 0),
                                         stop=(c == DCH - 1))
                    for j in range(NKCH):
                        nc.tensor.matmul(psk[j], lhsT=WK[:, c, j * P:(j + 1) * P],
                                         rhs=HTS[:, sb, c, :], start=(c == 0),
                                         stop=(c == DCH - 1))
                # fast ACT drains free the 6 banks; RoPE runs on DVE from
                # bf16 SBUF copies (2x mode) off the PE critical path
                qts = []
                for ps in psq + psk:
                    t = pqt.tile([P, SBLK], BF16, name="qt")
                    nc.scalar.copy(t, ps)
                    qts.append(t)

                def rope_pair(b0, b1, out0, out1):
                    c0 = CS[:, sb, 0, 0, :]; c1 = CS[:, sb, 0, 1, :]
                    s0 = CS[:, sb, 1, 0, :]; s1 = CS[:, sb, 1, 1, :]
                    t1 = ptmp.tile([P, SBLK], BF16, name="t")
                    t2 = ptmp.tile([P, SBLK], BF16, name="t")
                    nc.vector.tensor_mul(t1, b0, c0)
                    nc.vector.tensor_mul(t2, b1, s0)
                    nc.vector.tensor_sub(out0, t1, t2)
                    t3 = ptmp.tile([P, SBLK], BF16, name="t")
                    t4 = ptmp.tile([P, SBLK], BF16, name="t")
                    nc.vector.tensor_mul(t3, b1, c1)
                    nc.vector.tensor_mul(t4, b0, s1)
                    nc.vector.tensor_add(out1, t3, t4)

                for h in range(HEADS_PER_CORE):
                    rope_pair(qts[2 * h], qts[2 * h + 1],
                              QT[:, 2 * h, ssl], QT[:, 2 * h + 1, ssl])
                rope_pair(qts[NQCH], qts[NQCH + 1],
                          KT[:, 0, ssl], KT[:, 1, ssl])

                # direct V chains (PE work that covers the drains above)
                for si in range(SBLK // P):
                    # full-bank tile ([P,SBLK] f32 = 2KB) so two ppv bufs can
                    # never share a PSUM bank (PE-write + DVE-read collision)
                    psv = ppv.tile([P, SBLK], F32, name="pv")
                    for c in range(DCH):
                        nc.tensor.matmul(psv[:, :HD],
                                         lhsT=HTS[:, sb, c, si * P:(si + 1) * P],
                                         rhs=WV[:, c, :], start=(c == 0),
                                         stop=(c == DCH - 1))
                    # si 0/1 drain on ACT: the DVE is deep in this block's
                    # RoPE, and si 2/3's bank reuse must not wait for it
                    if si < 2:
                        nc.scalar.copy(VN[:, sb * (SBLK // P) + si, :],
                                       psv[:, :HD])
                    else:
                        nc.vector.tensor_copy(VN[:, sb * (SBLK // P) + si, :],
                                              psv[:, :HD])

        # ---- phase C+D: attention + norm + interleaved o_proj ---------
        with tc.tile_pool(name="pexp", bufs=17) as pexp, \
             tc.tile_pool(name="pou", bufs=4) as pou, \
             tc.tile_pool(name="pnrm", bufs=4) as pnrm, \
             tc.tile_pool(name="pdac", bufs=2) as pdac, \
             tc.tile_pool(name="pfin", bufs=4) as pfin, \
             tc.tile_pool(name="pmask", bufs=4) as pmask, \
             tc.tile_pool(name="ps_s", bufs=3, space="PSUM") as ps_s, \
             tc.tile_pool(name="ps_o", bufs=2, space="PSUM") as ps_o, \
             tc.tile_pool(name="ps_f", bufs=3, space="PSUM") as ps_f:

            def emit_norm(pend):
                ph, pqb, ou, dacc = pend
                # psb[m, q] = sum over partitions k of dacc[k, q], broadcast
                # to all 128 partitions in a single ONESB matmul
                psb = ps_f.tile([P, SBLK], F32, name="pf")
                nc.tensor.matmul(psb, lhsT=ONESB, rhs=dacc,
                                 start=True, stop=True)
                rbcf = pnrm.tile([P, SBLK], F32, name="rbc")
                nc.vector.reciprocal_approx_fast(out=rbcf, in_=psb)
                pqsl = slice(pqb * SBLK, (pqb + 1) * SBLK)
                for dvc in range(2):
                    nc.vector.tensor_mul(OUTN[:, 2 * ph + dvc, pqsl],
                                         ou[dvc], rbcf)

            def emit_oproj_quarter(qb):
                for st in range(4 * qb, 4 * qb + 4):
                    stsl = slice(st * P, (st + 1) * P)
                    for nbp in range(NSBLK // 2):
                        # two 512-wide psf accumulations drain into one
                        # 1024-wide bf16 tile -> 2KB DMA lines, half the bytes
                        fsb = pfin.tile([P, 2 * SBLK], BF16, name="fsb")
                        for half in range(2):
                            nb = 2 * nbp + half
                            psf = ps_f.tile([P, SBLK], F32, name="pf")
                            for dvc in range(NQCH):
                                nc.tensor.matmul(psf, lhsT=OUTN[:, dvc, stsl],
                                                 rhs=WO[:, dvc, nb * SBLK:(nb + 1) * SBLK],
                                                 start=(dvc == 0), stop=(dvc == NQCH - 1))
                            dst = fsb[:, half * SBLK:(half + 1) * SBLK]
                            if half == 0:
                                nc.vector.tensor_copy(dst, psf)
                            else:
                                nc.scalar.copy(dst, psf)
                        nc.sync.dma_start(
                            out=outp[stsl, 2 * nbp * SBLK:(2 * nbp + 2) * SBLK],
                            in_=fsb)

            pending = None          # (h, qb, ou, dacc) awaiting norm
            pending_oproj = None    # qb awaiting o_proj emission
            for h in range(HEADS_PER_CORE):
                for qb in range(NSBLK):
                    qsl = slice(qb * SBLK, (qb + 1) * SBLK)
                    klim = 4 * (qb + 1) if causal else NKC
                    pso = [ps_o.tile([P, SBLK], F32, name="po") for _ in range(2)]
                    exs = [None] * klim

                    # diag tiles only need q >= k: trim their q-range to
                    # [delta, 512) (the causal staircase handles the rest)
                    def qoff(i):
                        if causal and i >= 4 * qb:
                            return 128 * i - 512 * qb
                        return 0

                    widths = [SBLK - qoff(i) for i in range(klim)]

                    def emit_av(i):
                        ex, w = exs[i], widths[i]
                        for dvc in range(2):
                            nc.tensor.matmul(pso[dvc][:, SBLK - w:],
                                             lhsT=VN[:, i, dvc * P:(dvc + 1) * P],
                                             rhs=ex[:, :w], start=(i == 0),
                                             stop=(i == klim - 1))

                    # denominator accumulator: DVE adds of the exp tiles
                    # (i=0 is always full-width, so it initializes dacc)
                    dacc = pdac.tile([P, SBLK], BF16, name="dacc")

                    for i in range(klim):
                        qo, w = qoff(i), widths[i]
                        pss = ps_s.tile([P, SBLK], F32, name="ps")
                        for c in range(NKCH):
                            nc.tensor.matmul(pss[:, :w],
                                             lhsT=KT[:, c, i * P:(i + 1) * P],
                                             rhs=QT[:, 2 * h + c,
                                                    qb * SBLK + qo:(qb + 1) * SBLK],
                                             start=(c == 0), stop=(c == NKCH - 1))
                        if i == 1 and pending is not None:
                            emit_norm(pending)
                            pending = None
                        ex = pexp.tile([P, SBLK], BF16, name="ex")
                        nc.scalar.activation(ex[:, :w], pss[:, :w], EXP,
                                             scale=1.0 / 16.0)
                        if causal and i >= 4 * qb:
                            nc.vector.tensor_mul(ex[:, :w], ex[:, :w],
                                                 STAIR[:, 512:512 + w])
                        if not causal:
                            mt = pmask.tile([P, SBLK], BF16, name="mt")
                            nc.sync.dma_start(out=mt,
                                              in_=maskT[i * P:(i + 1) * P, qsl])
                            nc.vector.tensor_mul(ex, ex, mt)
                        exs[i] = ex
                        # denominator chain: fuse the first two full-width
                        # tiles into one add (skips the init copy); qb=0's
                        # i=1 tile is partial so it falls back to copy+add
                        if i == 0:
                            if widths[1] != SBLK:
                                nc.vector.tensor_copy(dacc, ex)
                        elif i == 1 and widths[1] == SBLK:
                            nc.vector.tensor_add(dacc, exs[0], ex)
                        else:
                            nc.vector.tensor_add(dacc[:, SBLK - w:],
                                                 dacc[:, SBLK - w:], ex[:, :w])
                        if i == 3 and pending_oproj is not None:
                            emit_oproj_quarter(pending_oproj)
                            pending_oproj = None
                        if i >= 2:
                            emit_av(i - 2)
                    emit_av(klim - 2)
                    emit_av(klim - 1)

                    last = (h == HEADS_PER_CORE - 1 and qb == NSBLK - 1)
                    if last:
                        # epilogue: normalize straight out of PSUM (no ou
                        # bounce). psb rides a score bank (all free now) so
                        # ps_f can double-buffer o_proj psf tiles; st12's two
                        # psf tiles run their dvc 0/1 (head 0, normalized one
                        # item ago) matmuls before any dvc 2/3, covering the
                        # recip+norm chain on the DVE.
                        psb = ps_s.tile([P, SBLK], F32, name="ps")
                        nc.tensor.matmul(psb, lhsT=ONESB, rhs=dacc,
                                         start=True, stop=True)
                        rbcf = pnrm.tile([P, SBLK], F32, name="rbc")
                        nc.vector.reciprocal_approx_fast(out=rbcf, in_=psb)
                        for dvc in range(2):
                            nc.vector.tensor_mul(OUTN[:, 2 * h + dvc, qsl],
                                                 pso[dvc], rbcf)
                        st = 4 * qb
                        stsl = slice(st * P, (st + 1) * P)
                        fsb = pfin.tile([P, 2 * SBLK], BF16, name="fsb")
                        held = []
                        for half in range(2):
                            psf = ps_f.tile([P, SBLK], F32, name="pf")
                            for dvc in range(2):
                                nc.tensor.matmul(psf, lhsT=OUTN[:, dvc, stsl],
                                                 rhs=WO[:, dvc, half * SBLK:(half + 1) * SBLK],
                                                 start=(dvc == 0), stop=False)
                            held.append(psf)
                        for half in range(2):
                            for dvc in range(2, NQCH):
                                nc.tensor.matmul(held[half], lhsT=OUTN[:, dvc, stsl],
                                                 rhs=WO[:, dvc, half * SBLK:(half + 1) * SBLK],
                                                 start=False, stop=(dvc == NQCH - 1))
                            dst = fsb[:, half * SBLK:(half + 1) * SBLK]
                            if half == 0:
                                nc.vector.tensor_copy(dst, held[half])
                            else:
                                nc.scalar.copy(dst, held[half])
                        nc.sync.dma_start(out=outp[stsl, 0:2 * SBLK], in_=fsb)
                        for st in range(4 * qb, 4 * qb + 4):
                            stsl = slice(st * P, (st + 1) * P)
                            for nbp in range(NSBLK // 2):
                                if st == 4 * qb and nbp == 0:
                                    continue
                                fsb = pfin.tile([P, 2 * SBLK], BF16, name="fsb")
                                final = (st == 4 * qb + 3 and nbp == 1)
                                for half in range(2):
                                    nb = 2 * nbp + half
                                    psf = ps_f.tile([P, SBLK], F32, name="pf")
                                    for dvc in range(NQCH):
                                        nc.tensor.matmul(psf, lhsT=OUTN[:, dvc, stsl],
                                                         rhs=WO[:, dvc, nb * SBLK:(nb + 1) * SBLK],
                                                         start=(dvc == 0), stop=(dvc == NQCH - 1))
                                    dst = fsb[:, half * SBLK:(half + 1) * SBLK]
                                    # the very last half drains on the DVE
                                    # (0.42us vs 0.7us ACT) to cut the tail
                                    if (half == 0) != final:
                                        nc.vector.tensor_copy(dst, psf)
                                    else:
                                        nc.scalar.copy(dst, psf)
                                nc.sync.dma_start(
                                    out=outp[stsl, 2 * nbp * SBLK:(2 * nbp + 2) * SBLK],
                                    in_=fsb)
                    else:
                        # immediate drains: free pso quickly (one on DVE,
                        # one on ACT to balance engine load)
                        ou = [pou.tile([P, SBLK], BF16, name="ou") for _ in range(2)]
                        nc.vector.tensor_copy(ou[0], pso[0])
                        nc.scalar.copy(ou[1], pso[1])
                        pending = (h, qb, ou, dacc)
                        if h == 1:
                            pending_oproj = qb

    nc.finalize()
    return nc


def _get_nc(causal: bool):
    key = bool(causal)
    if key not in _BUILD_CACHE:
        _BUILD_CACHE[key] = _build(causal)
    return _BUILD_CACHE[key]


def _rope_tables(position_ids_b):
    # cos/sin packed [P, NSBLK, 2, NKCH, SBLK]: s-block-major, one DMA per
    # block delivers both tables for that block
    pos = np.asarray(position_ids_b, dtype=np.float64)
    inv = 1.0 / (ROPE_BASE ** (np.arange(0, HD, 2, dtype=np.float64) / HD))
    f = pos[:, None] * inv[None, :]            # [S, HD/2]
    emb = np.concatenate([f, f], axis=1)       # [S, HD]
    cosT = np.cos(emb).T.astype(ml_dtypes.bfloat16)   # [HD, S]
    sinT = np.sin(emb).T.astype(ml_dtypes.bfloat16)
    def pack(x):
        return x.reshape(NKCH, P, NSBLK, SBLK).transpose(1, 2, 0, 3)
    return np.ascontiguousarray(
        np.stack([pack(cosT), pack(sinT)], axis=2))    # [P,NSBLK,2,NKCH,SBLK]


def _is_causal(attention_mask):
    m = np.asarray(attention_mask)
    if m.shape != (B, 1, S, S):
        return False
    tri = np.tril(np.ones((S, S), dtype=bool))
    canon = np.where(tri, np.float32(0.0), np.float32(-1e9))
    return all(np.array_equal(m[b, 0], canon) for b in range(B))


_ONESB_NP = np.ones((P, P), dtype=ml_dtypes.bfloat16)


def _stair():
    # multiplicative staircase: stair[p, j] = 1 if (j - 512) >= p else 0
    j = np.arange(2 * SBLK)[None, :] - SBLK
    p = np.arange(P)[:, None]
    return np.where(j >= p, 1.0, 0.0).astype(ml_dtypes.bfloat16)


def kernel(hidden_state, attention_mask, position_ids, Wq, Wk, Wv, Wo,
           _trace=False, _tmpdir=None):
    global LAST_EXEC_TIME_NS
    hidden_state = np.asarray(hidden_state, dtype=np.float32)
    Wq = np.asarray(Wq, dtype=np.float32)
    Wk = np.asarray(Wk, dtype=np.float32)
    Wv = np.asarray(Wv, dtype=np.float32)
    Wo = np.asarray(Wo, dtype=np.float32)

    causal = _is_causal(attention_mask)
    nc = _get_nc(causal)

    stair = _stair() if causal else None
    _WKP = []
    for W in (Wk, Wv):
        wb = W.astype(ml_dtypes.bfloat16)
        _WKP.append(np.ascontiguousarray(wb.reshape(DCH, P, HD).transpose(1, 0, 2)))
    per_batch = {}
    for b in range(B):
        hTb = hidden_state[b].T.astype(ml_dtypes.bfloat16)    # [D, S]
        hTp = np.ascontiguousarray(
            hTb.reshape(DCH, P, NSBLK, SBLK).transpose(1, 2, 0, 3))
        csb = _rope_tables(position_ids[b])
        mb = None
        if not causal:
            mb = np.ascontiguousarray(
                np.exp(np.asarray(attention_mask, dtype=np.float64)[b, 0].T)
            ).astype(ml_dtypes.bfloat16)
        per_batch[b] = (hTp, csb, mb)

    in_maps = []
    for core in range(8):
        b = core // 4
        hp = core % 4
        hTp, csb, mb = per_batch[b]
        wq_s = Wq[:, hp * DQ:(hp + 1) * DQ].astype(ml_dtypes.bfloat16)
        im = {
            "hTp": hTp,
            "onesb": _ONESB_NP,
            "wqp": np.ascontiguousarray(
                wq_s.reshape(DCH, P, DQ).transpose(1, 0, 2)),
            "wkp": _WKP[0],
            "wvp": _WKP[1],
            "wo": np.ascontiguousarray(
                Wo[hp * DQ:(hp + 1) * DQ, :]).astype(ml_dtypes.bfloat16),
            "csb": csb,
        }
        if causal:
            im["stair"] = stair
        else:
            im["emaskT"] = mb
        in_maps.append(im)

    res = run_bass_kernel_spmd(nc, in_maps, core_ids=list(range(8)),
                               trace=_trace, tmpdir=_tmpdir)
    LAST_EXEC_TIME_NS = res.exec_time_ns

    out = np.empty((B, S, D), dtype=np.float32)
    for b in range(B):
        acc = res.results[4 * b]["out_partial"].astype(np.float32)
        for hp in range(1, 4):
            acc = acc + res.results[4 * b + hp]["out_partial"].astype(np.float32)
        out[b] = acc
    return out


# revision 28
# speedup vs baseline: 1.0049x; 1.0049x over previous
"""GemmaAttention (B=2, S=2048, D=2048, H=8, KV=1, HD=256) on 8 trn2 NeuronCores.

Sharding: DP=2 over batch x TP=4 over head-pairs. Core c handles batch c//4 and
heads {2*(c%4), 2*(c%4)+1}. Each core computes its partial o_proj output
(row-parallel Wo); the host sums the 4 partials per batch (the all-reduce is
folded into the host-side unshard).

All matmuls run in bf16 (fp32 PSUM accumulate): fp32r streams at ~half the
bf16 column rate on the PE (389ns vs 213ns per N=512 matmul), so bf16 nearly
doubles tensor-engine throughput. rel-err budget is 2e-2; bf16 lands ~5e-3.

Dataflow per core:
  phase A (projections, per 512-col s-block):
    QT[dq,s], KT[dk,s] accumulate c-major over 16 D-chunks (6 PSUM banks),
    drained via ACT copy (psum->bf16 SBUF, frees banks fast) then RoPE on DVE
    in bf16 (2x mode). V[s,dv] computed directly (lhsT = hT chunk) in 128-row
    chains on 2 more banks -- this PE work covers the RoPE drain latency so
    the PE never idles at s-block boundaries. hT and cos/sin are packed
    s-block-major so each block's DMA completes before the PE needs it (the
    previous [P,DCH,S-512] packing interleaved blocks 1-3 within each DMA, so
    block 1 only landed when the whole 6MB stream did -> 8us PE stall).
  phase C (attention, per (head, q-block) item, software-pipelined):
    scoresT[k,q] = KT_chunk.T @ QT with a skew-2 pipeline: S(i) issues two
    iterations ahead of DEN(i)/AV(i) so the ACT exp (+DVE causal staircase)
    latency is hidden behind PE work. Denominators accumulate on the DVE
    (bf16 adds of the exp tiles into dacc), freeing the PE of the old
    ONEC-matmul reduction; one ONESB matmul per item then reduces+broadcasts
    dacc to all 128 partitions in a single 512-col pass. Per-item
    normalization is deferred one item (reciprocal_approx_fast + DVE scale)
    and o_proj quarters are injected into the next item's PE stream.
"""

import numpy as np
import ml_dtypes

import concourse.bass as bass
import concourse.tile as tile
import concourse.mybir as mybir
from concourse import bacc
from concourse.bass_utils import run_bass_kernel_spmd

P = 128
B, S, D = 2, 2048, 2048
H, KV, HD = 8, 1, 256
ROPE_BASE = 10000.0

HEADS_PER_CORE = 2
DQ = HEADS_PER_CORE * HD          # 512 q-dims per core
DCH = D // P                      # 16 contraction chunks
SBLK = 512                        # s-tile for projection rhs / q-tile
NSBLK = S // SBLK                 # 4
NKC = S // P                      # 16 key chunks
NQCH = DQ // P                    # 4 QT partition chunks
NKCH = HD // P                    # 2 KT partition chunks

F32 = mybir.dt.float32
BF16 = mybir.dt.bfloat16
EXP = mybir.ActivationFunctionType.Exp

LAST_EXEC_TIME_NS = None

_BUILD_CACHE = {}


def _build(causal: bool):
    nc = bacc.Bacc()

    # packed layouts: hTp/csb are s-block-major so each 512-col block's data
    # is one contiguous >=4KB-per-partition DMA (1KB lines cap the DMA ring
    # at ~150 GB/s; 2-3KB+ lines reach ~300-390 GB/s) and blocks arrive in
    # the order the PE consumes them
    hTp = nc.declare_dram_parameter("hTp", [P, NSBLK, DCH, SBLK], BF16,
                                    isOutput=False)
    wqp = nc.declare_dram_parameter("wqp", [P, DCH, DQ], BF16, isOutput=False)
    wkp = nc.declare_dram_parameter("wkp", [P, DCH, HD], BF16, isOutput=False)
    wvp = nc.declare_dram_parameter("wvp", [P, DCH, HD], BF16, isOutput=False)
    wo = nc.declare_dram_parameter("wo", [DQ, D], BF16, isOutput=False)
    csb = nc.declare_dram_parameter("csb", [P, NSBLK, 2, NKCH, SBLK], BF16,
                                    isOutput=False)
    onesb = nc.declare_dram_parameter("onesb", [P, P], BF16, isOutput=False)
    if causal:
        stair = nc.declare_dram_parameter("stair", [P, 2 * SBLK], BF16,
                                          isOutput=False)
    else:
        maskT = nc.declare_dram_parameter("emaskT", [S, S], BF16, isOutput=False)
    outp = nc.declare_dram_parameter("out_partial", [S, D], BF16, isOutput=True)

    from contextlib import ExitStack
    with tile.TileContext(nc) as tc, ExitStack() as ctx:
        pq = ctx.enter_context(tc.tile_pool(name="pq", bufs=1))
        QT = pq.tile([P, NQCH, S], BF16, name="QT")
        KT = pq.tile([P, NKCH, S], BF16, name="KT")
        VN = pq.tile([P, NKC, HD], BF16, name="VN")
        OUTN = pq.tile([P, NQCH, S], BF16, name="OUTN")
        WO = pq.tile([P, NQCH, D], BF16, name="WO")
        ONESB = pq.tile([P, P], BF16, name="ONESB")
        if causal:
            STAIR = pq.tile([P, 2 * SBLK], BF16, name="STAIR")

        # ---- phase A: projections + RoPE -----------------------------
        # pcs/pqt/ptmp live in the outer scope: the phase-A pool close then
        # only waits on PE-side reads (HTS/WQ), not on block 3's RoPE still
        # running on the DVE, so phase C's first scores start immediately
        pcs = ctx.enter_context(tc.tile_pool(name="pcs", bufs=1))
        pqt = ctx.enter_context(tc.tile_pool(name="pqt", bufs=8))
        ptmp = ctx.enter_context(tc.tile_pool(name="ptmp", bufs=6))
        with tc.tile_pool(name="pw", bufs=1) as pw, \
             tc.tile_pool(name="pht", bufs=1) as pht, \
             tc.tile_pool(name="ppqk", bufs=6, space="PSUM") as ppqk, \
             tc.tile_pool(name="ppv", bufs=2, space="PSUM") as ppv:
            WQ = pw.tile([P, DCH, DQ], BF16, name="WQ")
            WK = pw.tile([P, DCH, HD], BF16, name="WK")
            WV = pw.tile([P, DCH, HD], BF16, name="WV")

            HTS = pht.tile([P, NSBLK, DCH, SBLK], BF16, name="ht")
            CS = pcs.tile([P, NSBLK, 2, NKCH, SBLK], BF16, name="cs")
            for sb in range(NSBLK):
                if sb == 0:
                    # demand-ordered start: chunk 0/1 stream as singles so
                    # the first matmul issues ASAP, then 4-chunk groups (4KB
                    # lines) interleaved with the weight groups. Fewer DMAs
                    # matter: each dma_start costs ~650ns of issue time and
                    # hw queues throttle at 3-4 outstanding, so a long small-
                    # DMA stream delays block 1's issue past its deadline.
                    # WV streams late: the V chains only run at the end of
                    # the block (~35us), so V weights never displace the
                    # chunk-critical hT/WQ/WK bytes
                    nc.sync.dma_start(out=HTS[:, 0, 0:1, :],
                                      in_=hTp[:, 0, 0:1, :])
                    nc.sync.dma_start(out=WQ[:, 0:1, :], in_=wqp[:, 0:1, :])
                    nc.sync.dma_start(out=WK[:, 0:8, :], in_=wkp[:, 0:8, :])
                    nc.sync.dma_start(out=HTS[:, 0, 1:2, :],
                                      in_=hTp[:, 0, 1:2, :])
                    nc.sync.dma_start(out=WQ[:, 1:2, :], in_=wqp[:, 1:2, :])
                    nc.sync.dma_start(out=HTS[:, 0, 2:4, :],
                                      in_=hTp[:, 0, 2:4, :])
                    nc.sync.dma_start(out=WQ[:, 2:4, :], in_=wqp[:, 2:4, :])
                    for g in range(3):
                        gs = slice(4 * g + 4, 4 * g + 8)
                        nc.sync.dma_start(out=HTS[:, 0, gs, :],
                                          in_=hTp[:, 0, gs, :])
                        nc.sync.dma_start(out=WQ[:, gs, :], in_=wqp[:, gs, :])
                        if g == 0:
                            nc.sync.dma_start(out=WK[:, 8:16, :],
                                              in_=wkp[:, 8:16, :])
                        if g == 1:
                            nc.sync.dma_start(out=WV[:, 0:8, :],
                                              in_=wvp[:, 0:8, :])
                        if g == 2:
                            nc.sync.dma_start(out=WV[:, 8:16, :],
                                              in_=wvp[:, 8:16, :])
                    nc.sync.dma_start(out=CS[:, 0], in_=csb[:, 0])
                    # blocks 1-3 follow in block-major order, cos/sin after
                    # each block's hT so every block is fully resident before
                    # the PE reaches it
                    for b in range(1, NSBLK):
                        nc.sync.dma_start(out=HTS[:, b], in_=hTp[:, b])
                        nc.sync.dma_start(out=CS[:, b], in_=csb[:, b])
                        if b == 1:
                            nc.sync.dma_start(out=ONESB, in_=onesb[:, :])
                            if causal:
                                nc.sync.dma_start(out=STAIR, in_=stair[:, :])
                if sb == NSBLK - 1:
                    for cc in range(NQCH):
                        # o_proj weights are first needed deep into phase C;
                        # stream them late so they never delay hT/cos/sin
                        nc.sync.dma_start(out=WO[:, cc, :],
                                          in_=wo[cc * P:(cc + 1) * P, :])

                # Q/K accumulation, c-major (DMA-friendly: each ht used
                # 6x right after it lands)
                ssl = slice(sb * SBLK, (sb + 1) * SBLK)
                psq = [ppqk.tile([P, SBLK], F32, name="pp") for _ in range(NQCH)]
                psk = [ppqk.tile([P, SBLK], F32, name="pp") for _ in range(NKCH)]
                for c in range(DCH):
                    for i in range(NQCH):
                        nc.tensor.matmul(psq[i], lhsT=WQ[:, c, i * P:(i + 1) * P],
                                         rhs=HTS[:, sb, c, :], start=(c == 0),
                                         stop=(c == DCH - 1))
                    for j in range(NKCH):
                        nc.tensor.matmul(psk[j], lhsT=WK[:, c, j * P:(j + 1) * P],
                                         rhs=HTS[:, sb, c, :], start=(c == 0),
                                         stop=(c == DCH - 1))
                # fast ACT drains free the 6 banks; RoPE runs on DVE from
                # bf16 SBUF copies (2x mode) off the PE critical path
                qts = []
                for ps in psq + psk:
                    t = pqt.tile([P, SBLK], BF16, name="qt")
                    nc.scalar.copy(t, ps)
                    qts.append(t)

                def rope_pair(b0, b1, out0, out1):
                    c0 = CS[:, sb, 0, 0, :]; c1 = CS[:, sb, 0, 1, :]
                    s0 = CS[:, sb, 1, 0, :]; s1 = CS[:, sb, 1, 1, :]
                    t1 = ptmp.tile([P, SBLK], BF16, name="t")
                    t2 = ptmp.tile([P, SBLK], BF16, name="t")
                    nc.vector.tensor_mul(t1, b0, c0)
                    nc.vector.tensor_mul(t2, b1, s0)
                    nc.vector.tensor_sub(out0, t1, t2)
                    t3 = ptmp.tile([P, SBLK], BF16, name="t")
                    t4 = ptmp.tile([P, SBLK], BF16, name="t")
                    nc.vector.tensor_mul(t3, b1, c1)
                    nc.vector.tensor_mul(t4, b0, s1)
                    nc.vector.tensor_add(out1, t3, t4)

                for h in range(HEADS_PER_CORE):
                    rope_pair(qts[2 * h], qts[2 * h + 1],
                              QT[:, 2 * h, ssl], QT[:, 2 * h + 1, ssl])
                rope_pair(qts[NQCH], qts[NQCH + 1],
                          KT[:, 0, ssl], KT[:, 1, ssl])

                # direct V chains (PE work that covers the drains above)
                for si in range(SBLK // P):
                    # full-bank tile ([P,SBLK] f32 = 2KB) so two ppv bufs can
                    # never share a PSUM bank (PE-write + DVE-read collision)
                    psv = ppv.tile([P, SBLK], F32, name="pv")
                    for c in range(DCH):
                        nc.tensor.matmul(psv[:, :HD],
                                         lhsT=HTS[:, sb, c, si * P:(si + 1) * P],
                                         rhs=WV[:, c, :], start=(c == 0),
                                         stop=(c == DCH - 1))
                    # si 0/1 drain on ACT: the DVE is deep in this block's
                    # RoPE, and si 2/3's bank reuse must not wait for it
                    if si < 2:
                        nc.scalar.copy(VN[:, sb * (SBLK // P) + si, :],
                                       psv[:, :HD])
                    else:
                        nc.vector.tensor_copy(VN[:, sb * (SBLK // P) + si, :],
                                              psv[:, :HD])

        # ---- phase C+D: attention + norm + interleaved o_proj ---------
        with tc.tile_pool(name="pexp", bufs=17) as pexp, \
             tc.tile_pool(name="pou", bufs=4) as pou, \
             tc.tile_pool(name="pnrm", bufs=4) as pnrm, \
             tc.tile_pool(name="pdac", bufs=2) as pdac, \
             tc.tile_pool(name="pfin", bufs=4) as pfin, \
             tc.tile_pool(name="pmask", bufs=4) as pmask, \
             tc.tile_pool(name="ps_s", bufs=3, space="PSUM") as ps_s, \
             tc.tile_pool(name="ps_o", bufs=2, space="PSUM") as ps_o, \
             tc.tile_pool(name="ps_f", bufs=3, space="PSUM") as ps_f:

            def emit_norm(pend):
                ph, pqb, ou, dacc = pend
                # psb[m, q] = sum over partitions k of dacc[k, q], broadcast
                # to all 128 partitions in a single ONESB matmul
                psb = ps_f.tile([P, SBLK], F32, name="pf")
                nc.tensor.matmul(psb, lhsT=ONESB, rhs=dacc,
                                 start=True, stop=True)
                rbcf = pnrm.tile([P, SBLK], F32, name="rbc")
                nc.vector.reciprocal_approx_fast(out=rbcf, in_=psb)
                pqsl = slice(pqb * SBLK, (pqb + 1) * SBLK)
                for dvc in range(2):
                    nc.vector.tensor_mul(OUTN[:, 2 * ph + dvc, pqsl],
                                         ou[dvc], rbcf)

            def emit_oproj_quarter(qb):
                for st in range(4 * qb, 4 * qb + 4):
                    stsl = slice(st * P, (st + 1) * P)
                    for nbp in range(NSBLK // 2):
                        # two 512-wide psf accumulations drain into one
                        # 1024-wide bf16 tile -> 2KB DMA lines, half the bytes
                        fsb = pfin.tile([P, 2 * SBLK], BF16, name="fsb")
                        for half in range(2):
                            nb = 2 * nbp + half
                            psf = ps_f.tile([P, SBLK], F32, name="pf")
                            for dvc in range(NQCH):
                                nc.tensor.matmul(psf, lhsT=OUTN[:, dvc, stsl],
                                                 rhs=WO[:, dvc, nb * SBLK:(nb + 1) * SBLK],
                                                 start=(dvc == 0), stop=(dvc == NQCH - 1))
                            dst = fsb[:, half * SBLK:(half + 1) * SBLK]
                            if half == 0:
                                nc.vector.tensor_copy(dst, psf)
                            else:
                                nc.scalar.copy(dst, psf)
                        nc.sync.dma_start(
                            out=outp[stsl, 2 * nbp * SBLK:(2 * nbp + 2) * SBLK],
                            in_=fsb)

            pending = None          # (h, qb, ou, dacc) awaiting norm
            pending_oproj = None    # qb awaiting o_proj emission
            for h in range(HEADS_PER_CORE):
                for qb in range(NSBLK):
                    qsl = slice(qb * SBLK, (qb + 1) * SBLK)
                    klim = 4 * (qb + 1) if causal else NKC
                    pso = [ps_o.tile([P, SBLK], F32, name="po") for _ in range(2)]
                    exs = [None] * klim

                    # diag tiles only need q >= k: trim their q-range to
                    # [delta, 512) (the causal staircase handles the rest)
                    def qoff(i):
                        if causal and i >= 4 * qb:
                            return 128 * i - 512 * qb
                        return 0

                    widths = [SBLK - qoff(i) for i in range(klim)]

                    def emit_av(i):
                        ex, w = exs[i], widths[i]
                        for dvc in range(2):
                            nc.tensor.matmul(pso[dvc][:, SBLK - w:],
                                             lhsT=VN[:, i, dvc * P:(dvc + 1) * P],
                                             rhs=ex[:, :w], start=(i == 0),
                                             stop=(i == klim - 1))

                    # denominator accumulator: DVE adds of the exp tiles
                    # (i=0 is always full-width, so it initializes dacc)
                    dacc = pdac.tile([P, SBLK], BF16, name="dacc")

                    for i in range(klim):
                        qo, w = qoff(i), widths[i]
                        pss = ps_s.tile([P, SBLK], F32, name="ps")
                        for c in range(NKCH):
                            nc.tensor.matmul(pss[:, :w],
                                             lhsT=KT[:, c, i * P:(i + 1) * P],
                                             rhs=QT[:, 2 * h + c,
                                                    qb * SBLK + qo:(qb + 1) * SBLK],
                                             start=(c == 0), stop=(c == NKCH - 1))
                        if i == 1 and pending is not None:
                            emit_norm(pending)
                            pending = None
                        ex = pexp.tile([P, SBLK], BF16, name="ex")
                        nc.scalar.activation(ex[:, :w], pss[:, :w], EXP,
                                             scale=1.0 / 16.0)
                        if causal and i >= 4 * qb:
                            nc.vector.tensor_mul(ex[:, :w], ex[:, :w],
                                                 STAIR[:, 512:512 + w])
                        if not causal:
                            mt = pmask.tile([P, SBLK], BF16, name="mt")
                            nc.sync.dma_start(out=mt,
                                              in_=maskT[i * P:(i + 1) * P, qsl])
                            nc.vector.tensor_mul(ex, ex, mt)
                        exs[i] = ex
                        # denominator chain: fuse the first two full-width
                        # tiles into one add (skips the init copy); qb=0's
                        # i=1 tile is partial so it falls back to copy+add
                        if i == 0:
                            if widths[1] != SBLK:
                                nc.vector.tensor_copy(dacc, ex)
                        elif i == 1 and widths[1] == SBLK:
                            nc.vector.tensor_add(dacc, exs[0], ex)
                        else:
                            nc.vector.tensor_add(dacc[:, SBLK - w:],
                                                 dacc[:, SBLK - w:], ex[:, :w])
                        if i == 3 and pending_oproj is not None:
                            emit_oproj_quarter(pending_oproj)
                            pending_oproj = None
                        if i >= 2:
                            emit_av(i - 2)
                    emit_av(klim - 2)
                    emit_av(klim - 1)

                    last = (h == HEADS_PER_CORE - 1 and qb == NSBLK - 1)
                    if last:
                        # epilogue: normalize straight out of PSUM (no ou
                        # bounce). psb rides a score bank (all free now) so
                        # ps_f can double-buffer o_proj psf tiles; st12's two
                        # psf tiles run their dvc 0/1 (head 0, normalized one
                        # item ago) matmuls before any dvc 2/3, covering the
                        # recip+norm chain on the DVE.
                        psb = ps_s.tile([P, SBLK], F32, name="ps")
                        nc.tensor.matmul(psb, lhsT=ONESB, rhs=dacc,
                                         start=True, stop=True)
                        rbcf = pnrm.tile([P, SBLK], F32, name="rbc")
                        nc.vector.reciprocal_approx_fast(out=rbcf, in_=psb)
                        for dvc in range(2):
                            nc.vector.tensor_mul(OUTN[:, 2 * h + dvc, qsl],
                                                 pso[dvc], rbcf)
                        st = 4 * qb
                        stsl = slice(st * P, (st + 1) * P)
                        fsb = pfin.tile([P, 2 * SBLK], BF16, name="fsb")
                        held = []
                        for half in range(2):
                            psf = ps_f.tile([P, SBLK], F32, name="pf")
                            for dvc in range(2):
                                nc.tensor.matmul(psf, lhsT=OUTN[:, dvc, stsl],
                                                 rhs=WO[:, dvc, half * SBLK:(half + 1) * SBLK],
                                                 start=(dvc == 0), stop=False)
                            held.append(psf)
                        for half in range(2):
                            for dvc in range(2, NQCH):
                                nc.tensor.matmul(held[half], lhsT=OUTN[:, dvc, stsl],
                                                 rhs=WO[:, dvc, half * SBLK:(half + 1) * SBLK],
                                                 start=False, stop=(dvc == NQCH - 1))
                            dst = fsb[:, half * SBLK:(half + 1) * SBLK]
                            if half == 0:
                                nc.vector.tensor_copy(dst, held[half])
                            else:
                                nc.scalar.copy(dst, held[half])
                        nc.sync.dma_start(out=outp[stsl, 0:2 * SBLK], in_=fsb)
                        for st in range(4 * qb, 4 * qb + 4):
                            stsl = slice(st * P, (st + 1) * P)
                            for nbp in range(NSBLK // 2):
                                if st == 4 * qb and nbp == 0:
                                    continue
                                fsb = pfin.tile([P, 2 * SBLK], BF16, name="fsb")
                                final = (st == 4 * qb + 3 and nbp == 1)
                                for half in range(2):
                                    nb = 2 * nbp + half
                                    psf = ps_f.tile([P, SBLK], F32, name="pf")
                                    for dvc in range(NQCH):
                                        nc.tensor.matmul(psf, lhsT=OUTN[:, dvc, stsl],
                                                         rhs=WO[:, dvc, nb * SBLK:(nb + 1) * SBLK],
                                                         start=(dvc == 0), stop=(dvc == NQCH - 1))
                                    dst = fsb[:, half * SBLK:(half + 1) * SBLK]
                                    # the very last half drains on the DVE
                                    # (0.42us vs 0.7us ACT) to cut the tail
                                    if (half == 0) != final:
                                        nc.vector.tensor_copy(dst, psf)
                                    else:
                                        nc.scalar.copy(dst, psf)
                                nc.sync.dma_start(
                                    out=outp[stsl, 2 * nbp * SBLK:(2 * nbp + 2) * SBLK],
                                    in_=fsb)
                    else:
                        # immediate drains: free pso quickly (one on DVE,
                        # one on ACT to balance engine load)
                        ou = [pou.tile([P, SBLK], BF16, name="ou") for _ in range(2)]
                        nc.vector.tensor_copy(ou[0], pso[0])
                        nc.scalar.copy(ou[1], pso[1])
                        pending = (h, qb, ou, dacc)
                        if h == 1:
                            pending_oproj = qb

    nc.finalize()
    return nc


def _get_nc(causal: bool):
    key = bool(causal)
    if key not in _BUILD_CACHE:
        _BUILD_CACHE[key] = _build(causal)
    return _BUILD_CACHE[key]


def _rope_tables(position_ids_b):
    # cos/sin packed [P, NSBLK, 2, NKCH, SBLK]: s-block-major, one DMA per
    # block delivers both tables for that block
    pos = np.asarray(position_ids_b, dtype=np.float64)
    inv = 1.0 / (ROPE_BASE ** (np.arange(0, HD, 2, dtype=np.float64) / HD))
    f = pos[:, None] * inv[None, :]            # [S, HD/2]
    emb = np.concatenate([f, f], axis=1)       # [S, HD]
    cosT = np.cos(emb).T.astype(ml_dtypes.bfloat16)   # [HD, S]
    sinT = np.sin(emb).T.astype(ml_dtypes.bfloat16)
    def pack(x):
        return x.reshape(NKCH, P, NSBLK, SBLK).transpose(1, 2, 0, 3)
    return np.ascontiguousarray(
        np.stack([pack(cosT), pack(sinT)], axis=2))    # [P,NSBLK,2,NKCH,SBLK]


def _is_causal(attention_mask):
    m = np.asarray(attention_mask)
    if m.shape != (B, 1, S, S):
        return False
    tri = np.tril(np.ones((S, S), dtype=bool))
    canon = np.where(tri, np.float32(0.0), np.float32(-1e9))
    return all(np.array_equal(m[b, 0], canon) for b in range(B))


_ONESB_NP = np.ones((P, P), dtype=ml_dtypes.bfloat16)


def _stair():
    # multiplicative staircase: stair[p, j] = 1 if (j - 512) >= p else 0
    j = np.arange(2 * SBLK)[None, :] - SBLK
    p = np.arange(P)[:, None]
    return np.where(j >= p, 1.0, 0.0).astype(ml_dtypes.bfloat16)


def kernel(hidden_state, attention_mask, position_ids, Wq, Wk, Wv, Wo,
           _trace=False, _tmpdir=None):
    global LAST_EXEC_TIME_NS
    hidden_state = np.asarray(hidden_state, dtype=np.float32)
    Wq = np.asarray(Wq, dtype=np.float32)
    Wk = np.asarray(Wk, dtype=np.float32)
    Wv = np.asarray(Wv, dtype=np.float32)
    Wo = np.asarray(Wo, dtype=np.float32)

    causal = _is_causal(attention_mask)
    nc = _get_nc(causal)

    stair = _stair() if causal else None
    _WKP = []
    for W in (Wk, Wv):
        wb = W.astype(ml_dtypes.bfloat16)
        _WKP.append(np.ascontiguousarray(wb.reshape(DCH, P, HD).transpose(1, 0, 2)))
    per_batch = {}
    for b in range(B):
        hTb = hidden_state[b].T.astype(ml_dtypes.bfloat16)    # [D, S]
        hTp = np.ascontiguousarray(
            hTb.reshape(DCH, P, NSBLK, SBLK).transpose(1, 2, 0, 3))
        csb = _rope_tables(position_ids[b])
        mb = None
        if not causal:
            mb = np.ascontiguousarray(
                np.exp(np.asarray(attention_mask, dtype=np.float64)[b, 0].T)
            ).astype(ml_dtypes.bfloat16)
        per_batch[b] = (hTp, csb, mb)

    in_maps = []
    for core in range(8):
        b = core // 4
        hp = core % 4
        hTp, csb, mb = per_batch[b]
        wq_s = Wq[:, hp * DQ:(hp + 1) * DQ].astype(ml_dtypes.bfloat16)
        im = {
            "hTp": hTp,
            "onesb": _ONESB_NP,
            "wqp": np.ascontiguousarray(
                wq_s.reshape(DCH, P, DQ).transpose(1, 0, 2)),
            "wkp": _WKP[0],
            "wvp": _WKP[1],
            "wo": np.ascontiguousarray(
                Wo[hp * DQ:(hp + 1) * DQ, :]).astype(ml_dtypes.bfloat16),
            "csb": csb,
        }
        if causal:
            im["stair"] = stair
        else:
            im["emaskT"] = mb
        in_maps.append(im)

    res = run_bass_kernel_spmd(nc, in_maps, core_ids=list(range(8)),
                               trace=_trace, tmpdir=_tmpdir)
    LAST_EXEC_TIME_NS = res.exec_time_ns

    out = np.empty((B, S, D), dtype=np.float32)
    for b in range(B):
        acc = res.results[4 * b]["out_partial"].astype(np.float32)
        for hp in range(1, 4):
            acc = acc + res.results[4 * b + hp]["out_partial"].astype(np.float32)
        out[b] = acc
    return out
